# revision 1
# baseline (speedup 1.0000x reference)
"""AssignmentSimilarityNet GNN message-passing kernel for 8 Trainium2
NeuronCores.

Sharding: track (A) dimension split across 8 cores (32 tracks each).
Edge tensors, track embeds and messages-to-A stay local; messages-to-B
(sum over A) are all-reduced each step; MLP weights replicated.

Layouts (per core, feature-major: features on SBUF partitions):
  - edge/pair tensors: (128 feat, 8192 pairs), pairs ordered a-major
    (pair = a*256 + b, a in [0,32), b in [0,256))
  - compute dtype: float32r (full PE rate, ~1e-4 rounding) with f32 PSUM
  - logits per step: transposed-c2 matmuls (pairs on partitions) so the
    PSUM->SBUF copy is wide, then one DMA into the (8, 8192) output.
"""
import numpy as np

A = 256          # tracks
B = 256          # current detections
AL = A // 8      # tracks per core (32)
REID = 512
D = 128          # ND == ED
STEPS = 8
NP = AL * B      # pairs per core (8192)
CH = 512         # pair chunk (2 a-groups x 256 b)
NCH = NP // CH   # 16

_BUILD_CACHE = {}


def _build():
    if "nc" in _BUILD_CACHE:
        return _BUILD_CACHE["nc"]
    import concourse.bacc as bacc
    import concourse.mybir as mybir
    import concourse.tile as tile

    F32 = mybir.dt.float32
    F32R = mybir.dt.float32r
    BF16 = mybir.dt.bfloat16
    AF = mybir.ActivationFunctionType
    ALU = mybir.AluOpType

    nc = bacc.Bacc(None, target_bir_lowering=False)

    def din(name, shape):
        return nc.dram_tensor(name, shape, F32, kind="ExternalInput")

    tfT = din("tfT", [REID, AL])
    trkf = din("trkf", [AL, REID])
    cfT = din("cfT", [REID, B])
    curf = din("curf", [B, REID])
    trkg = din("trkg", [AL, 5])
    curg = din("curg", [B, 5])
    wlinT = din("wlinT", [REID, D])
    blin = din("blin", [D, 1])
    wein1T = din("wein1T", [6, D])
    bein1 = din("bein1", [D, 1])
    wein2T = din("wein2T", [D, D])
    bein2 = din("bein2", [D, 1])
    we1T = din("we1T", [4 * D, D])
    be1 = din("be1", [D, 1])
    we2T = din("we2T", [D, D])
    be2 = din("be2", [D, 1])
    wn1T = din("wn1T", [2 * D, D])
    bn1 = din("bn1", [D, 1])
    wn2T = din("wn2T", [D, D])
    bn2 = din("bn2", [D, 1])
    wc1T = din("wc1T", [D, D])
    bc1 = din("bc1", [D, 1])
    wc2p = din("wc2p", [D, 2])
    bc2 = din("bc2", [D, 1])
    out = nc.dram_tensor("out", [STEPS, NP], F32, kind="ExternalOutput")

    with tile.TileContext(nc) as tc:
        with (
            tc.tile_pool(name="const", bufs=1) as cp,
            tc.tile_pool(name="state", bufs=1) as st,
            tc.tile_pool(name="work", bufs=1) as wk,
            tc.tile_pool(name="p1", bufs=3, space="PSUM") as pp1,
            tc.tile_pool(name="p2", bufs=2, space="PSUM") as pp2,
            tc.tile_pool(name="p3", bufs=2, space="PSUM") as pp3,
            tc.tile_pool(name="plg", bufs=1, space="PSUM") as plgp,
            tc.tile_pool(name="dram", bufs=1, space="DRAM") as dr,
        ):
            # ---------------- feature loads ----------------
            tf_t = st.tile([128, 4 * AL], F32R)       # 4 K-tiles of (128, 32)
            cf_t = st.tile([128, 4 * B], F32R)        # 4 K-tiles of (128, 256)
            tf_s = wk.tile([128, 4 * AL], F32, tag="wstage", bufs=2)
            cf_s = wk.tile([128, 4 * B], F32, tag="wstage", bufs=2)
            for j in range(4):
                nc.sync.dma_start(tf_s[:, AL * j:AL * (j + 1)],
                                  tfT[128 * j:128 * (j + 1), :])
                nc.sync.dma_start(cf_s[:, B * j:B * (j + 1)],
                                  cfT[128 * j:128 * (j + 1), :])
            nc.vector.tensor_copy(tf_t[:], tf_s[:])
            nc.vector.tensor_copy(cf_t[:], cf_s[:])
            trkf_t = wk.tile([AL, REID], F32)
            nc.sync.dma_start(trkf_t[:], trkf[:])
            curf_t0 = wk.tile([128, REID], F32)
            curf_t1 = wk.tile([128, REID], F32)
            nc.sync.dma_start(curf_t0[:], curf[0:128, :])
            nc.sync.dma_start(curf_t1[:], curf[128:256, :])
            trkg_t = wk.tile([AL, 5], F32)
            nc.sync.dma_start(trkg_t[:], trkg[:])
            curg_t0 = wk.tile([128, 5], F32)
            curg_t1 = wk.tile([128, 5], F32)
            nc.sync.dma_start(curg_t0[:], curg[0:128, :])
            nc.sync.dma_start(curg_t1[:], curg[128:256, :])

            # ---------------- weight / bias loads ----------------
            we1_t = cp.tile([128, 4 * D], F32R)
            wlin_t = cp.tile([128, 4 * D], F32R)
            wn1_t = cp.tile([128, 2 * D], F32R)
            we1_s = wk.tile([128, 4 * D], F32, tag="wstage", bufs=2)
            wlin_s = wk.tile([128, 4 * D], F32, tag="wstage", bufs=2)
            wn1_s = wk.tile([128, 2 * D], F32, tag="wstage", bufs=2)
            for j in range(4):
                nc.sync.dma_start(we1_s[:, 128 * j:128 * (j + 1)],
                                  we1T[128 * j:128 * (j + 1), :])
                nc.sync.dma_start(wlin_s[:, 128 * j:128 * (j + 1)],
                                  wlinT[128 * j:128 * (j + 1), :])
            for j in range(2):
                nc.sync.dma_start(wn1_s[:, 128 * j:128 * (j + 1)],
                                  wn1T[128 * j:128 * (j + 1), :])
            nc.vector.tensor_copy(we1_t[:], we1_s[:])
            nc.vector.tensor_copy(wlin_t[:], wlin_s[:])
            nc.vector.tensor_copy(wn1_t[:], wn1_s[:])
            wein1_t = cp.tile([6, D], F32R)
            wein2_t = cp.tile([128, D], F32R)
            we2_t = cp.tile([128, D], F32R)
            wn2_t = cp.tile([128, D], F32R)
            wc1_t = cp.tile([128, D], F32R)
            wc2p_t = cp.tile([128, 2], F32R)
            for dst, src in [(wein1_t, wein1T), (wein2_t, wein2T),
                             (we2_t, we2T), (wn2_t, wn2T), (wc1_t, wc1T),
                             (wc2p_t, wc2p)]:
                s = wk.tile(list(dst.shape), F32, name=f"ws_{src.name}",
                            tag="wstage", bufs=2)
                nc.sync.dma_start(s[:], src[:])
                nc.vector.tensor_copy(dst[:], s[:])
            biases = {}
            for nm, src in [("blin", blin), ("bein1", bein1), ("bein2", bein2),
                            ("be1", be1), ("be2", be2), ("bn1", bn1),
                            ("bn2", bn2), ("bc1", bc1), ("bc2", bc2)]:
                t = cp.tile([128, 1], F32, name=f"b_{nm}")
                nc.sync.dma_start(t[:], src[:])
                biases[nm] = t

            # ---------------- reid norms ----------------
            sq_t = wk.tile([AL, REID], F32, tag="sq", bufs=2)
            nc.vector.tensor_mul(sq_t[:], trkf_t[:], trkf_t[:])
            sst = wk.tile([AL, 1], F32)
            nc.vector.tensor_reduce(sst[:], sq_t[:], mybir.AxisListType.X, ALU.add)
            rt = wk.tile([AL, 1], F32)
            nc.vector.reciprocal(rt[:], sst[:])
            inv_t = wk.tile([AL, 1], F32)
            nc.scalar.activation(inv_t[:], rt[:], AF.Sqrt)

            invc = []
            for i, ct in enumerate((curf_t0, curf_t1)):
                sq_c = wk.tile([128, REID], F32, name=f"sq_c{i}", tag="sq", bufs=2)
                nc.vector.tensor_mul(sq_c[:], ct[:], ct[:])
                ssc = wk.tile([128, 1], F32, name=f"ssc{i}")
                nc.vector.tensor_reduce(ssc[:], sq_c[:], mybir.AxisListType.X,
                                        ALU.add)
                rc = wk.tile([128, 1], F32, name=f"rc{i}")
                nc.vector.reciprocal(rc[:], ssc[:])
                ic = wk.tile([128, 1], F32, name=f"ic{i}")
                nc.scalar.activation(ic[:], rc[:], AF.Sqrt)
                invc.append(ic)

            # ---------------- current-side geometry -> bcast rows ----------
            # rows of cstage: 0 xb, 1 yb, 2 hb, 3 ln hb, 4 ln wb, 5 tb, 6 invc
            cstage = dr.tile([7, B], F32)
            for i, gt in enumerate((curg_t0, curg_t1)):
                half = slice(128 * i, 128 * (i + 1))
                cg = wk.tile([128, 7], F32, name=f"cg{i}")
                nc.vector.tensor_add(cg[:, 0:1], gt[:, 0:1], gt[:, 2:3])
                nc.vector.tensor_scalar_mul(cg[:, 0:1], cg[:, 0:1], 0.5)
                nc.vector.tensor_add(cg[:, 1:2], gt[:, 1:2], gt[:, 3:4])
                nc.vector.tensor_scalar_mul(cg[:, 1:2], cg[:, 1:2], 0.5)
                nc.vector.tensor_sub(cg[:, 2:3], gt[:, 3:4], gt[:, 1:2])
                wb = wk.tile([128, 1], F32, name=f"wb{i}")
                nc.vector.tensor_sub(wb[:], gt[:, 2:3], gt[:, 0:1])
                nc.scalar.activation(cg[:, 3:4], cg[:, 2:3], AF.Ln)
                nc.scalar.activation(cg[:, 4:5], wb[:], AF.Ln)
                nc.vector.tensor_copy(cg[:, 5:6], gt[:, 4:5])
                nc.vector.tensor_copy(cg[:, 6:7], invc[i][:])
                nc.sync.dma_start(cstage[:, half].transpose((1, 0)), cg[:])
            bcall = wk.tile([AL, 7 * B], F32)
            nc.sync.dma_start(
                bcall[:], cstage[:, :].partition_broadcast(AL)
                .rearrange("p r b -> p (r b)"))
            bc = {nm: bcall[:, B * r:B * (r + 1)]
                  for r, nm in enumerate(["xb", "yb", "hb", "lnhb",
                                          "lnwb", "tb", "invc"])}

            # ---------------- track-side geometry scalars ----------------
            xt = wk.tile([AL, 1], F32)
            nc.vector.tensor_add(xt[:], trkg_t[:, 0:1], trkg_t[:, 2:3])
            nc.vector.tensor_scalar_mul(xt[:], xt[:], 0.5)
            yt = wk.tile([AL, 1], F32)
            nc.vector.tensor_add(yt[:], trkg_t[:, 1:2], trkg_t[:, 3:4])
            nc.vector.tensor_scalar_mul(yt[:], yt[:], 0.5)
            ht = wk.tile([AL, 1], F32)
            nc.vector.tensor_sub(ht[:], trkg_t[:, 3:4], trkg_t[:, 1:2])
            wt = wk.tile([AL, 1], F32)
            nc.vector.tensor_sub(wt[:], trkg_t[:, 2:3], trkg_t[:, 0:1])
            lnht = wk.tile([AL, 1], F32)
            nc.scalar.activation(lnht[:], ht[:], AF.Ln)
            lnwt = wk.tile([AL, 1], F32)
            nc.scalar.activation(lnwt[:], wt[:], AF.Ln)

            # ---------------- edge features (AL, B) each ----------------
            den = wk.tile([AL, B], F32)
            nc.vector.tensor_scalar_add(den[:], bc["hb"][:], ht[:, 0:1])
            rden = wk.tile([AL, B], F32)
            nc.vector.reciprocal(rden[:], den[:])

            feats = []
            f0 = wk.tile([AL, B], F32, name="f_x")
            nc.vector.tensor_scalar(f0[:], bc["xb"][:], xt[:, 0:1], 2.0,
                                    ALU.subtract, ALU.mult)
            nc.vector.tensor_mul(f0[:], f0[:], rden[:])
            feats.append(f0)
            f1 = wk.tile([AL, B], F32, name="f_y")
            nc.vector.tensor_scalar(f1[:], bc["yb"][:], yt[:, 0:1], 2.0,
                                    ALU.subtract, ALU.mult)
            nc.vector.tensor_mul(f1[:], f1[:], rden[:])
            feats.append(f1)
            f2 = wk.tile([AL, B], F32, name="f_w")
            nc.vector.tensor_scalar(f2[:], bc["lnwb"][:], -1.0, lnwt[:, 0:1],
                                    ALU.mult, ALU.add)
            feats.append(f2)
            f3 = wk.tile([AL, B], F32, name="f_h")
            nc.vector.tensor_scalar(f3[:], bc["lnhb"][:], -1.0, lnht[:, 0:1],
                                    ALU.mult, ALU.add)
            feats.append(f3)
            f4 = wk.tile([AL, B], F32, name="f_t")
            nc.vector.tensor_scalar_sub(f4[:], bc["tb"][:], trkg_t[:, 4:5])
            feats.append(f4)

            # dist_reid = 1 - gram * inv_t * inv_c
            pg = pp3.tile([AL, B], F32, tag="p3")
            for j in range(4):
                nc.tensor.matmul(pg[:], tf_t[:, AL * j:AL * (j + 1)],
                                 cf_t[:, B * j:B * (j + 1)],
                                 start=(j == 0), stop=(j == 3))
            f5 = wk.tile([AL, B], F32, name="f_d")
            nc.vector.tensor_scalar(f5[:], pg[:], inv_t[:, 0:1], None,
                                    ALU.mult)
            nc.vector.tensor_mul(f5[:], f5[:], bc["invc"][:])
            nc.scalar.activation(f5[:], f5[:], AF.Copy, bias=1.0, scale=-1.0)
            feats.append(f5)

            # ---------------- transpose features -> efT (6, 8192) ----------
            ef_stage = dr.tile([6, NP], F32R)
            for f, t in enumerate(feats):
                fr = wk.tile([AL, B], F32R, name=f"fr{f}")
                nc.vector.tensor_copy(fr[:], t[:])
                nc.sync.dma_start(
                    ef_stage[f:f + 1, :].rearrange("o (a b) -> (o a) b", a=AL),
                    fr[:])
            upds = [st.tile([128, NP], F32R, name="updA"),
                    st.tile([128, NP], F32R, name="updB")]
            efT_t = upds[0][0:6, :]
            nc.sync.dma_start(efT_t, ef_stage[:])

            # ---------------- fixed_edge = mlp2(edge_feats) ----------------
            fixedT = st.tile([128, NP], F32R)
            for c in range(NCH):
                sl = slice(CH * c, CH * (c + 1))
                p1 = pp1.tile([128, CH], F32, tag="p1")
                nc.tensor.matmul(p1[:], wein1_t[:], efT_t[:, sl],
                                 start=True, stop=True)
                h = wk.tile([128, CH], F32R, tag="h1", bufs=3)
                if c % 2 == 0:
                    nc.scalar.activation(h[:], p1[:], AF.Relu,
                                         bias=biases["bein1"][:, 0:1])
                else:
                    nc.vector.tensor_scalar(h[:], p1[:],
                                            biases["bein1"][:, 0:1], 0.0,
                                            ALU.add, ALU.max)
                p2 = pp2.tile([128, CH], F32, tag="p2")
                nc.tensor.matmul(p2[:], wein2_t[:], h[:], start=True, stop=True)
                if c % 2 == 0:
                    nc.vector.tensor_scalar(fixedT[:, sl], p2[:],
                                            biases["bein2"][:, 0:1], 0.0,
                                            ALU.add, ALU.max)
                else:
                    nc.scalar.activation(fixedT[:, sl], p2[:], AF.Relu,
                                         bias=biases["bein2"][:, 0:1])

            # ---------------- initial node embeds ----------------
            pt = pp2.tile([128, AL], F32, tag="p2")
            for j in range(4):
                nc.tensor.matmul(pt[:], wlin_t[:, 128 * j:128 * (j + 1)],
                                 tf_t[:, AL * j:AL * (j + 1)],
                                 start=(j == 0), stop=(j == 3))
            te = [st.tile([128, AL], F32R, name="teA"),
                  st.tile([128, AL], F32R, name="teB")]
            nc.scalar.activation(te[0][:], pt[:], AF.Relu,
                                 bias=biases["blin"][:, 0:1])
            pc = pp2.tile([128, B], F32, tag="p2")
            for j in range(4):
                nc.tensor.matmul(pc[:], wlin_t[:, 128 * j:128 * (j + 1)],
                                 cf_t[:, B * j:B * (j + 1)],
                                 start=(j == 0), stop=(j == 3))
            ce = [st.tile([128, B], F32R, name="ceA"),
                  st.tile([128, B], F32R, name="ceB")]
            nc.scalar.activation(ce[0][:], pc[:], AF.Relu,
                                 bias=biases["blin"][:, 0:1])

            # ---------------- message-passing steps ----------------
            # Unified software pipeline: each step's edge-MLP loop also
            # carries the PREVIOUS step's classifier (AR-independent), so
            # the PE never drains while the all-gather is in flight.
            pend = {}

            def finish_curr(ce_dst):
                mb_out_p, msgb_f2 = pend.pop("ar")
                for q in range(2):
                    nc.gpsimd.dma_start(msgb_f2[64 * q:64 * (q + 1), :],
                                        mb_out_p[64 * q:64 * (q + 1), :])
                pc1 = pp2.tile([128, B], F32, tag="p2")
                nc.tensor.matmul(pc1[:], wn1_t[:, 0:128], pend.pop("ce_prev"),
                                 start=True, stop=False)
                nc.tensor.matmul(pc1[:], wn1_t[:, 128:256], msgb_f2[:],
                                 start=False, stop=True)
                cn1 = wk.tile([128, B], F32R, tag="cn1", bufs=2)
                nc.vector.tensor_scalar(cn1[:], pc1[:], biases["bn1"][:, 0:1],
                                        0.0, ALU.add, ALU.max)
                pc2 = pp2.tile([128, B], F32, tag="p2")
                nc.tensor.matmul(pc2[:], wn2_t[:], cn1[:], start=True,
                                 stop=True)
                nc.vector.tensor_scalar(ce_dst, pc2[:], biases["bn2"][:, 0:1],
                                        0.0, ALU.add, ALU.max)

            def open3(c, u_prev, te_cur):
                p1 = pp1.tile([128, CH], F32, tag="p1")
                sl = slice(CH * c, CH * (c + 1))
                nc.tensor.matmul(p1[:], we1_t[:, 384:512], fixedT[:, sl],
                                 start=True, stop=False)
                nc.tensor.matmul(p1[:], we1_t[:, 256:384], u_prev[:, sl],
                                 start=False, stop=False)
                nc.tensor.matmul(
                    p1[:], we1_t[:, 0:128],
                    te_cur[:, 2 * c:2 * c + 2].to_broadcast((128, 2, B)),
                    start=False, stop=False)
                return p1

            def close_ce(p1, ce_cur):
                nc.tensor.matmul(
                    p1[:], we1_t[:, 128:256],
                    ce_cur[:, :].to_broadcast((128, B, 2))
                    .transpose((0, 2, 1)), start=False, stop=True)

            def h1_relu(p1):
                h1 = wk.tile([128, CH], F32R, tag="h1", bufs=3)
                nc.scalar.activation(h1[:], p1[:], AF.Relu,
                                     bias=biases["be1"][:, 0:1])
                return h1

            def do_tail(c, h1, u_cur, msga, msgb):
                sl = slice(CH * c, CH * (c + 1))
                p2 = pp2.tile([128, CH], F32, tag="p2")
                nc.tensor.matmul(p2[:], we2_t[:], h1[:], start=True, stop=True)
                nc.scalar.activation(u_cur[:, sl], p2[:], AF.Relu,
                                     bias=biases["be2"][:, 0:1])
                if c == 0:
                    nc.vector.tensor_add(msgb[:],
                                         u_cur[:, 0:B].bitcast(F32),
                                         u_cur[:, B:2 * B].bitcast(F32))
                else:
                    for g in range(2):
                        nc.vector.tensor_add(
                            msgb[:], msgb[:],
                            u_cur[:, CH * c + B * g:CH * c + B * (g + 1)]
                            .bitcast(F32))
                nc.vector.tensor_reduce(
                    msga[:, 2 * c:2 * c + 2],
                    u_cur[:, sl].bitcast(F32)
                    .rearrange("p (a b) -> p a b", a=2),
                    mybir.AxisListType.X, ALU.add)

            def pass2_piece(ctx, i):
                u_src = ctx["u"]
                sl = slice(CH * i, CH * (i + 1))
                p3 = pp3.tile([128, CH], F32, tag="p3")
                nc.tensor.matmul(p3[:], wc1_t[:], u_src[:, sl],
                                 start=True, stop=True)
                hc = wk.tile([128, CH], F32R, tag="hc", bufs=2)
                if i % 2 == 0:
                    nc.scalar.activation(hc[:], p3[:], AF.Relu,
                                         bias=biases["bc1"][:, 0:1])
                else:
                    nc.vector.tensor_scalar(hc[:], p3[:],
                                            biases["bc1"][:, 0:1], 0.0,
                                            ALU.add, ALU.max)
                for pc_, ph_ in ctx["hc_pend"]:
                    for j in range(4):
                        col = 2 * (4 * pc_ + j)
                        nc.tensor.matmul(ctx["plg"][:, col:col + 2],
                                         ph_[:, 128 * j:128 * (j + 1)],
                                         wc2p_t[:], start=True, stop=True)
                ctx["hc_pend"] = [(i, hc)]

            def pass2_drain(ctx, kprev):
                for pc_, ph_ in ctx["hc_pend"]:
                    for j in range(4):
                        col = 2 * (4 * pc_ + j)
                        nc.tensor.matmul(ctx["plg"][:, col:col + 2],
                                         ph_[:, 128 * j:128 * (j + 1)],
                                         wc2p_t[:], start=True, stop=True)
                ctx["hc_pend"] = []
                lg_s = wk.tile([128, NCH * 4], F32, tag="lgs", bufs=2)
                nc.vector.tensor_scalar(lg_s[:], ctx["plg"][:, 0:2 * NCH * 4:2],
                                        biases["bc2"][:, 0:1], None, ALU.add)
                nc.sync.dma_start(
                    out[kprev:kprev + 1, :]
                    .rearrange("o (c p) -> (o p) c", p=128), lg_s[:])

            prev_ctx = None
            for k in range(STEPS):
                u_prev = fixedT if k == 0 else upds[(k + 1) % 2]
                u_cur = upds[k % 2]
                te_cur, te_nxt = te[k % 2], te[(k + 1) % 2]
                ce_cur, ce_nxt = ce[k % 2], ce[(k + 1) % 2]

                msga = wk.tile([128, AL], F32, tag="msga", bufs=2)
                msgb = wk.tile([128, B], F32, tag="msgb", bufs=2)

                h1q = []
                if k > 0:
                    part = [(c, open3(c, u_prev, te_cur)) for c in range(3)]
                    nxt_open = 3
                else:
                    part = []
                    nxt_open = 0
                for i in range(16):
                    if prev_ctx is not None:
                        pass2_piece(prev_ctx, i)
                    if k > 0 and i == 0:
                        finish_curr(ce_cur[:, :])
                    if k > 0 and i == 1:
                        for c, p1 in part:
                            close_ce(p1, ce_cur)
                        for c, p1 in part:
                            h1q.append((c, h1_relu(p1)))
                        part = []
                    if i >= (2 if k > 0 else 0) and nxt_open < NCH:
                        c = nxt_open
                        nxt_open += 1
                        p1 = open3(c, u_prev, te_cur)
                        close_ce(p1, ce_cur)
                        h1q.append((c, h1_relu(p1)))
                    while len(h1q) > 2:
                        tc_, th_ = h1q.pop(0)
                        do_tail(tc_, th_, u_cur, msga, msgb)
                while nxt_open < NCH:
                    c = nxt_open
                    nxt_open += 1
                    p1 = open3(c, u_prev, te_cur)
                    close_ce(p1, ce_cur)
                    h1q.append((c, h1_relu(p1)))
                    while len(h1q) > 2:
                        tc_, th_ = h1q.pop(0)
                        do_tail(tc_, th_, u_cur, msga, msgb)
                for tc_, th_ in h1q:
                    do_tail(tc_, th_, u_cur, msga, msgb)
                h1q = []

                # -- all-reduce msg_b --
                mb_in = dr.tile([128, B], F32, tag="mbin", bufs=2)
                mb_out = dr.tile([128, B], F32, tag="mbout", bufs=2,
                                 addr_space="Shared")
                for q in range(4):
                    nc.sync.dma_start(mb_in[32 * q:32 * (q + 1), :],
                                      msgb[32 * q:32 * (q + 1), :])
                nc.gpsimd.collective_compute(
                    "AllReduce", mybir.AluOpType.add,
                    replica_groups=[list(range(8))],
                    ins=[mb_in.opt()], outs=[mb_out.opt()])
                msgb_f = wk.tile([128, B], F32R, tag="msgbf", bufs=2)
                pend["ar"] = (mb_out, msgb_f)
                pend["ce_prev"] = ce_cur[:, :]

                # wind down previous classifier, then track update
                if prev_ctx is not None:
                    pass2_drain(prev_ctx, k - 1)
                msga_r = wk.tile([128, AL], F32R, tag="msgar", bufs=2)
                nc.vector.tensor_copy(msga_r[:], msga[:])
                pt1 = pp2.tile([128, AL], F32, tag="p2")
                nc.tensor.matmul(pt1[:], wn1_t[:, 0:128], te_cur[:],
                                 start=True, stop=False)
                nc.tensor.matmul(pt1[:], wn1_t[:, 128:256], msga_r[:],
                                 start=False, stop=True)
                tn1 = wk.tile([128, AL], F32R, tag="tn1", bufs=2)
                nc.vector.tensor_scalar(tn1[:], pt1[:], biases["bn1"][:, 0:1],
                                        0.0, ALU.add, ALU.max)
                pt2 = pp2.tile([128, AL], F32, tag="p2")
                nc.tensor.matmul(pt2[:], wn2_t[:], tn1[:], start=True,
                                 stop=True)
                nc.vector.tensor_scalar(te_nxt[:], pt2[:], biases["bn2"][:, 0:1],
                                        0.0, ALU.add, ALU.max)

                p_lg_n = plgp.tile([128, 2 * NCH * 4], F32, tag="plg",
                                   name="p_lg_n")
                prev_ctx = {"u": u_cur, "plg": p_lg_n, "hc_pend": []}

            # final step classifier wind-down
            for i in range(16):
                pass2_piece(prev_ctx, i)
            pass2_drain(prev_ctx, STEPS - 1)

    nc.finalize()
    _BUILD_CACHE["nc"] = nc
    return nc


def _make_in_maps(inputs):
    f32 = np.float32

    def c(x):
        return np.ascontiguousarray(np.asarray(x, dtype=f32))

    tf = c(inputs["track_features"])
    cf = c(inputs["current_features"])
    tb = c(inputs["track_boxes"])
    cb = c(inputs["current_boxes"])
    tt = c(inputs["track_time"]).reshape(-1, 1)
    ct = c(inputs["current_time"]).reshape(-1, 1)

    shared = {
        "cfT": c(cf.T),
        "curf": cf,
        "curg": c(np.concatenate([cb, ct], axis=1)),
        "wlinT": c(inputs["w_lin"].T),
        "blin": c(np.broadcast_to(inputs["b_lin"][:, None], (D, 1))),
        "wein1T": c(inputs["w_ein1"].T),
        "bein1": c(inputs["b_ein1"][:, None]),
        "wein2T": c(inputs["w_ein2"].T),
        "bein2": c(inputs["b_ein2"][:, None]),
        "we1T": c(inputs["w_e1"].T),
        "be1": c(inputs["b_e1"][:, None]),
        "we2T": c(inputs["w_e2"].T),
        "be2": c(inputs["b_e2"][:, None]),
        "wn1T": c(inputs["w_n1"].T),
        "bn1": c(inputs["b_n1"][:, None]),
        "wn2T": c(inputs["w_n2"].T),
        "bn2": c(inputs["b_n2"][:, None]),
        "wc1T": c(inputs["w_c1"].T),
        "bc1": c(inputs["b_c1"][:, None]),
        "wc2p": c(np.concatenate([inputs["w_c2"].T,
                                  np.zeros((D, 1), f32)], axis=1)),
        "bc2": c(np.broadcast_to(np.asarray(inputs["b_c2"], f32)
                                 .reshape(1, 1), (D, 1))),
    }
    in_maps = []
    for core in range(8):
        rows = slice(AL * core, AL * (core + 1))
        m = dict(shared)
        m["tfT"] = c(tf[rows].T)
        m["trkf"] = c(tf[rows])
        m["trkg"] = c(np.concatenate([tb[rows], tt[rows]], axis=1))
        in_maps.append(m)
    return in_maps


def run(trace=False, trace_cores=None, **inputs):
    from concourse.bass_utils import run_bass_kernel_spmd

    if trace:
        _install_ntff_hook()
    nc = _build()
    in_maps = _make_in_maps(inputs)
    res = run_bass_kernel_spmd(nc, in_maps, core_ids=list(range(8)),
                               trace=trace, trace_cores=trace_cores)
    full = np.empty((STEPS, A, B), np.float32)
    for core in range(8):
        full[:, AL * core:AL * (core + 1), :] = \
            res.results[core]["out"].reshape(STEPS, AL, B)
    return full, res


def kernel(**inputs):
    full, _ = run(trace=False, **inputs)
    return full


def _install_ntff_hook():
    import sys
    import types
    try:
        from antenv.axon_hooks import get_axon_ntff_profile_hook  # noqa: F401
        return
    except ImportError:
        pass
    import antenv
    from trn_agent_boot.trn_boot import _ntff_profile_via_ctypes

    mod = types.ModuleType("antenv.axon_hooks")
    holder = [_ntff_profile_via_ctypes("/opt/axon/libaxon_pjrt.so")]
    mod.get_axon_ntff_profile_hook = lambda: holder[0]
    mod.set_axon_ntff_profile_hook = lambda h: holder.__setitem__(0, h)
    sys.modules["antenv.axon_hooks"] = mod
    antenv.axon_hooks = mod

